# revision 1
# baseline (speedup 1.0000x reference)
"""Trainium2 Bass kernel for nn_DiscriminationLoss (segment_reduce), v4.

Per core (one image, data-parallel over batch):
  s[k, c] = sum_p pred[p, c] * [lab[p] == k], k = 1..8, plus counts n[k].
Pixels as [R=128, Q=3200]; superchunks of J=16 columns; per superchunk one
matmul accumulates acc[(c,j), (k,j')] into PSUM; host takes the j==j'
diagonal.  pred ships as fp8e4 (RNE; |s|~600 >> sigma=3 so the loss is
insensitive).  One-hot planes k=1..8 via DVE is_equal (counts via
accum_out).  A mid-stream span of DR_SC superchunks uses fp8 planes
(straight from uint8 labels) and DoubleRow matmuls -- 2 superchunks per
matmul at half the per-column cost -- to balance PE against DVE.  The
other spans use bf16 planes (DVE 4x) from per-group ACT-cast labels.
Groups are dependency-chained so the Tile scheduler cannot reorder them;
warmup matmuls into a scratch PSUM bank pre-ramp the PE.
"""

import numpy as np
from contextlib import ExitStack

import concourse.bass as bass  # noqa: F401
import concourse.tile as tile
from concourse import bacc, mybir
from concourse.bass_utils import run_bass_kernel_spmd
from concourse.tile_rust import add_dep_helper

B, C, H, W = 8, 8, 640, 640
P_PIX = H * W
R = 128
Q = P_PIX // R         # 3200
SIGMA = 3.0
J = 16
K = 8
M = C * J              # 128
N = K * J              # 128
NSC = Q // J           # 200

# (start_sc, n_sc, kind): "u8" = uint8-direct bf16 planes (no cast dep),
# "f8" = fp8 planes (uint8-direct) feeding DoubleRow matmuls,
# "bf" = bf16 planes from ACT-cast labels.
GROUPS = [
    (0, 20, "u8"),
    (20, 36, "u8"),
    (56, 48, "bf"),
    (104, 48, "bf"),
    (152, 44, "f8"),
    (196, 4, "bf"),
]
DR0, DR_SC = 152, 44   # DoubleRow span == the single f8 group (0 = disabled)
CHUNKS = [12, 16, 20, 24, 28, 32, 20, 24, 20, 4]
assert sum(CHUNKS) == NSC
assert sum(g[1] for g in GROUPS) == NSC
assert DR_SC == 0 or (DR0 in np.cumsum(CHUNKS)
                      and (DR0 + DR_SC) in np.cumsum(CHUNKS))

# ACT cast pieces (start_sc, n_sc) covering exactly the "bf" spans.
CASTS = [(56, 48), (104, 48), (196, 4)]
# Label DMA pieces after the group-0 piece (ACT queue), by sc.
LAB_PIECES = [(20, 36), (56, 72), (128, 72)]
N_WARMUP = 30
# Dummy matmuls inserted after the matmuls of these superchunk indices, to
# bridge PE idle seams (keeps the p-state ramp warm).
SEAM_DUMMIES = {19: 3, 55: 3}

_cached_nc = None


def _raw(h):
    return getattr(h, "ins", h)


def _build_program():
    nc = bacc.Bacc("TRN2", target_bir_lowering=False, debug=False,
                   enable_asserts=False, num_devices=B)
    pred_d = nc.dram_tensor("pred", [R, NSC, C, J], mybir.dt.float8e4,
                            kind="ExternalInput")
    lab_d = nc.dram_tensor("lab", [R, Q], mybir.dt.uint8,
                           kind="ExternalInput")
    out_d = nc.dram_tensor("out", [M, N], mybir.dt.float32,
                           kind="ExternalOutput")
    ng = len(GROUPS)
    cnt_d = nc.dram_tensor("cnt", [R, ng * K], mybir.dt.float32,
                           kind="ExternalOutput")

    with tile.TileContext(nc) as tc, ExitStack() as ctx:
        singles = ctx.enter_context(tc.tile_pool(name="singles", bufs=1))
        psum_pool = ctx.enter_context(
            tc.tile_pool(name="psum", bufs=1, space="PSUM"))

        pred_t = singles.tile([R, NSC, C, J], mybir.dt.float8e4)
        ohb = singles.tile([R, NSC, K, J], mybir.dt.bfloat16)  # bf planes
        oh8 = singles.tile([R, DR_SC, K, J], mybir.dt.float8e4)
        lab_u8 = singles.tile([R, Q], mybir.dt.uint8)
        lab_bf = singles.tile([R, Q], mybir.dt.bfloat16)
        cnt = singles.tile([R, ng * K], mybir.dt.float32)
        dw = singles.tile([R, M], mybir.dt.bfloat16)
        dr_ = singles.tile([R, N], mybir.dt.bfloat16)

        acc = psum_pool.tile([M, N], mybir.dt.float32)
        scratch = psum_pool.tile([M, N], mybir.dt.float32)

        pred_ap = pred_d.ap()
        lab_ap = lab_d.ap()

        # PE warmup (no data deps; fills the p-state ramp window)
        nc.vector.memset(dw[:], 0.0)
        nc.vector.memset(dr_[:], 0.0)
        for _ in range(N_WARMUP):
            nc.tensor.matmul(scratch[:, :], lhsT=dw[:], rhs=dr_[:],
                             start=True, stop=True)

        # Labels: group-0 piece first on the SP queue, rest on ACT queue.
        g0c = GROUPS[0][1] * J
        nc.sync.dma_start(out=lab_u8[:, :g0c], in_=lab_ap[:, :g0c])
        for s0, nsc in LAB_PIECES:
            q0, q1 = s0 * J, (s0 + nsc) * J
            nc.scalar.dma_start(out=lab_u8[:, q0:q1], in_=lab_ap[:, q0:q1])

        # ACT: per-span casts for the bf16 groups.
        for s0, nsc in CASTS:
            q0, q1 = s0 * J, (s0 + nsc) * J
            nc.scalar.copy(out=lab_bf[:, q0:q1], in_=lab_u8[:, q0:q1])

        # One-hot planes + counts; chain groups so the scheduler keeps order.
        prev = None
        for g, (s0, nsc, kind) in enumerate(GROUPS):
            q0, q1 = s0 * J, (s0 + nsc) * J
            if kind == "u8":
                # DVE-side cast: cheaper than 8 u8-direct is_equal passes
                # and avoids an ACT-cast dependency early in the chain
                h = nc.vector.tensor_copy(out=lab_bf[:, q0:q1],
                                          in_=lab_u8[:, q0:q1])
                if prev is not None:
                    add_dep_helper(_raw(h), _raw(prev), False,
                                   "serialize one-hot groups")
                prev = h
                lt = lab_bf[:, q0:q1]
            elif kind == "f8":
                lt = lab_u8[:, q0:q1]
            else:
                lt = lab_bf[:, q0:q1]
            if kind == "f8":
                oh_g = oh8[:, s0 - DR0 : s0 + nsc - DR0, :, :]
            else:
                oh_g = ohb[:, s0 : s0 + nsc, :, :]
            lt = lt.rearrange("r (s j) -> r s j", j=J)
            for k in range(1, K + 1):
                h = nc.vector.tensor_scalar(
                    out=oh_g[:, :, k - 1, :],
                    in0=lt[:, :, :],
                    scalar1=float(k),
                    scalar2=None,
                    op0=mybir.AluOpType.is_equal,
                    op1=mybir.AluOpType.add,
                    accum_out=cnt[:, g * K + k - 1 : g * K + k],
                )
                if prev is not None:
                    add_dep_helper(_raw(h), _raw(prev), False,
                                   "serialize one-hot groups")
                prev = h

        # pred stream + matmuls (DoubleRow pairs inside the f8 span)
        sc0 = 0
        for scc in CHUNKS:
            nc.sync.dma_start(out=pred_t[:, sc0 : sc0 + scc, :, :],
                              in_=pred_ap[:, sc0 : sc0 + scc, :, :])
            s = sc0
            while s < sc0 + scc:
                if DR_SC and DR0 <= s < DR0 + DR_SC:
                    t = s - DR0
                    nc.tensor.matmul(
                        acc[:, :],
                        lhsT=pred_t[:, s : s + 2, :, :],
                        rhs=oh8[:, t : t + 2, :, :],
                        start=False, stop=False,
                        perf_mode=mybir.MatmulPerfMode.DoubleRow,
                        skip_group_check=True,
                    )
                    s += 2
                else:
                    nc.tensor.matmul(
                        acc[:, :],
                        lhsT=pred_t[:, s, :, :],
                        rhs=ohb[:, s, :, :],
                        start=(s == 0), stop=(s == NSC - 1),
                        skip_group_check=True,
                    )
                    for _ in range(SEAM_DUMMIES.get(s, 0)):
                        nc.tensor.matmul(scratch[:, :], lhsT=dw[:], rhs=dr_[:],
                                         start=True, stop=True)
                    s += 1
            sc0 += scc

        nc.scalar.dma_start(out=cnt_d.ap()[:, :], in_=cnt[:])
        ot = singles.tile([M, N], mybir.dt.float32)
        nc.vector.tensor_copy(out=ot[:], in_=acc[:, :])
        nc.sync.dma_start(out=out_d.ap()[:, :], in_=ot[:])

    nc.compile()
    return nc


def _get_program():
    global _cached_nc
    if _cached_nc is None:
        _cached_nc = _build_program()
    return _cached_nc


def _make_in_maps(pred_similarities, kernel_mask_ndi_labels):
    import ml_dtypes

    pred = (
        np.asarray(pred_similarities, dtype=np.float32)
        .reshape(B, C, R, NSC, J)
        .astype(ml_dtypes.float8_e4m3fn)
    )
    predperm = np.ascontiguousarray(pred.transpose(0, 2, 3, 1, 4))
    lab = np.asarray(kernel_mask_ndi_labels).reshape(B, R, Q).astype(np.uint8)
    return [{"pred": predperm[b], "lab": lab[b]} for b in range(B)]


def _finalize(results):
    f_sigma = float(np.log(SIGMA**2 + 1.0))
    ng = len(GROUPS)
    total = 0.0
    for b in range(B):
        O = np.asarray(results[b]["out"], dtype=np.float64).reshape(C, J, K, J)
        s = np.einsum("cjkj->kc", O)                # [k, c], labels 1..8
        cnt = np.asarray(results[b]["cnt"], dtype=np.float64)
        n = cnt.reshape(R, ng, K).sum(axis=(0, 1))   # [k] exact counts
        present = np.nonzero(n > 0.5)[0]
        num_kernel = int(present.max()) + 1 if present.size else 0
        m = float(num_kernel)
        snorm = np.sqrt((s * s).sum(axis=1))
        f = np.log(np.maximum(SIGMA - snorm, 0.0) ** 2 + 1.0)
        valid = np.arange(1, K + 1) <= num_kernel
        per_kernel = float((n * (f - f_sigma))[valid].sum())
        num_pairs = m * (m - 1.0) * 0.5
        total += (m - 1.0) * per_kernel + num_pairs * (B * P_PIX) * f_sigma
    return np.asarray(total, dtype=np.float32)


def kernel(pred_similarities, kernel_mask_ndi_labels):
    nc = _get_program()
    in_maps = _make_in_maps(pred_similarities, kernel_mask_ndi_labels)
    last_err = None
    for attempt in range(4):
        try:
            res = run_bass_kernel_spmd(nc, in_maps, core_ids=list(range(B)))
            # materialize inside the try: device errors can surface lazily
            results = [
                {k: np.asarray(v) for k, v in r.items()} for r in res.results
            ]
            return _finalize(results)
        except Exception as e:  # noqa: BLE001 - retry transient device wedges
            last_err = e
            import time

            time.sleep(10 * (attempt + 1))
    raise last_err


def modeled_exec_time_ns():
    from concourse.timeline_sim import TimelineSim

    return TimelineSim(_get_program(), trace=False).simulate()



# revision 34
# speedup vs baseline: 1.1420x; 1.1420x over previous
"""Trainium2 Bass kernel for nn_DiscriminationLoss (segment_reduce), v5.

Per core (one image, data-parallel over batch): segment sums
  s[k, c] = sum_p pred[p, c] * [lab[p] == k], k = 1..8, plus counts n[k].

v5 key ideas (vs v4 baseline's one-hot + bf16/fp8 mix at 18.7us):
  * Hinge basis instead of one-hot: plane_i(p) = relu(lab[p] - (i+0.5)),
    i = 0..7.  Linear in the one-hot with an invertible 8x8 matrix
    T[i,k] = relu(k - i - 0.5) (k = 1..8); background (lab=0) maps to 0 in
    every plane.  The host solves T s~ = u after the fact.  Unlike
    is_equal, a hinge plane is a single pass on ANY engine:
      - DVE:  tensor_scalar(op0=subtract, op1=max)
      - ACT:  activation(Relu, bias=-(i+0.5))
      - Pool: gpsimd tensor_scalar
    so plane generation is split across all three (DVE ~4 planes,
    ACT ~2.5, Pool ~1.5), each comfortably under the DMA wall.
  * All planes fp8 (values i+0.5 are exact in e4m3) -> every matmul runs
    fp8 DoubleRow: PE work is 100 x 64cy = tiny.
  * Counts ride on accum_out of each plane instruction (free in the cost
    model, exact f32 sums of halves); A[i] = sum T[i,k] n_k, host solves.
  * One label DMA (big contiguous pieces, no sub-512B descriptor
    penalty), 8 pred chunk DMAs all >= 627ns so HWDGE pipelining never
    gaps the DMA engines.
  * Counts are copied into spare columns of the gram PSUM bank so ONE
    output DMA ships everything.

Cost model floor: preamble 0.7 + first-DMA stages 1.3 + DMA busy 10.5
(lab 1.14 + pred 9.10 + out 0.23) + tail (mm+sem+stages+xfer+sem+barrier
~3.0) ~= 15.3us.
"""

import numpy as np
from contextlib import ExitStack

import concourse.bass as bass  # noqa: F401
import concourse.tile as tile
from concourse import bacc, mybir
from concourse.bass_utils import run_bass_kernel_spmd
from concourse.tile_rust import add_dep_helper

B, C, H, W = 8, 8, 640, 640
P_PIX = H * W
R = 128
Q = P_PIX // R         # 3200
SIGMA = 3.0
J = 16
K = 8                  # number of hinge planes (= number of kernels)
M = C * J              # 128
N = K * J              # 128
NSC = Q // J           # 200

# pred DMA chunks (superchunks each): all even (DoubleRow pairs never
# straddle a chunk) and big enough that each transfer covers the next
# DMA instruction's 627ns HWDGE stage, so the DMA engines never gap.
# The tiny final chunk lets most of the last matmuls start one DMA-sem
# (900ns) earlier.
CHUNKS = [26, 26, 26, 26, 24, 24, 24, 20, 4]
assert sum(CHUNKS) == NSC

# label DMA pieces in Q columns (16 per superchunk); interleaved between
# the first pred chunks (big transfers cover the small pieces' HWDGE
# time), boundaries aligned with plane-span boundaries (sc 92 = col 1472).
LAB_PIECES = [(0, 1472), (1472, 3200)]
# DMA program: ("lab", piece_idx) / ("pred", chunk_idx) in stream order
DMA_ORDER = [("lab", 0), ("pred", 0), ("lab", 1)] + [
    ("pred", i) for i in range(1, len(CHUNKS))]

# plane assignment: ordered per engine: list of (engine, plane_i, sc0, sc1)
# DVE: planes 0-3 + tail of 6; ACT: planes 4,5 + middle of 6; Pool:
# plane 7 + head of 6.  Early superchunks get small spans on every engine
# so the matmul stream starts ~5us.
PLANE_ASSIGN = []
for s0, s1 in [(0, 26), (26, 52), (52, 92), (92, 146), (146, 200)]:
    for i in range(4):
        PLANE_ASSIGN.append(("dve", i, s0, s1))
PLANE_ASSIGN.append(("dve", 6, 146, 200))
ACT_PLAN = [(4, 0, 32), (5, 0, 32), (4, 32, 92), (5, 32, 92),
            (4, 92, 200), (5, 92, 200), (6, 92, 146)]
for i, s0, s1 in ACT_PLAN:
    PLANE_ASSIGN.append(("act", i, s0, s1))
POOL_PLAN = [(6, 0, 32), (7, 0, 32), (6, 32, 92), (7, 32, 92),
             (7, 92, 200)]
for i, s0, s1 in POOL_PLAN:
    PLANE_ASSIGN.append(("pool", i, s0, s1))

OUTW = 2 * N               # [gram | counts] in the single output

# sanity: every (plane, sc) covered exactly once
_cover = np.zeros((K, NSC), dtype=int)
for _, i, s0, s1 in PLANE_ASSIGN:
    _cover[i, s0:s1] += 1
assert (_cover == 1).all()

N_WARMUP = 0               # PE keeps pace even at mid p-state

_cached_nc = None


def _raw(h):
    return getattr(h, "ins", h)


def _build_program():
    nc = bacc.Bacc("TRN2", target_bir_lowering=False, debug=False,
                   enable_asserts=False, num_devices=B)
    pred_d = nc.dram_tensor("pred", [R, NSC, C, J], mybir.dt.float8e4,
                            kind="ExternalInput")
    lab_d = nc.dram_tensor("lab", [R, Q], mybir.dt.uint8,
                           kind="ExternalInput")
    # output shaped for kv_writeback: [batch=1, dhi=128, dho=1, n_ctx=OUTW]
    out_d = nc.dram_tensor("out", [1, R, 1, OUTW], mybir.dt.float32,
                           kind="ExternalOutput")

    with tile.TileContext(nc) as tc, ExitStack() as ctx:
        singles = ctx.enter_context(tc.tile_pool(name="singles", bufs=1))
        psum_pool = ctx.enter_context(
            tc.tile_pool(name="psum", bufs=1, space="PSUM"))

        pred_t = singles.tile([R, NSC, C, J], mybir.dt.float8e4)
        oh8 = singles.tile([R, NSC, K, J], mybir.dt.float8e4)
        lab_u8 = singles.tile([R, Q], mybir.dt.uint8)
        # output staging tile + an alias at the same bytes: the writeback
        # PREP reads the alias so Tile attaches no data deps to it (the
        # trigger is gated manually); writers use `ot`.
        ot_h = nc.alloc_sbuf_tensor("ot", [R, OUTW], mybir.dt.float32)
        ot_alias_h = nc.alloc_sbuf_tensor_at(
            "ot_alias", [R, OUTW], mybir.dt.float32,
            offset=nc.lookup_mloc(ot_h).addr)
        ot = ot_h.ap()
        # biases for the ACT planes (pass scalar directly on DVE/Pool);
        # bias_t[:, i] = -(i + 0.5) for the planes ACT owns, plus a zero
        # column for the table-warm dummy activation.
        act_planes = sorted({i for e, i, _, _ in PLANE_ASSIGN if e == "act"})
        bias_t = singles.tile([R, len(act_planes) + 1], mybir.dt.float32)
        bias_col = {}
        nc.gpsimd.memset(bias_t[:, 0:1], 0.0)
        for ci, i in enumerate(act_planes):
            bias_col[i] = ci + 1
            nc.gpsimd.memset(bias_t[:, ci + 1:ci + 2], -(float(i) + 0.5))

        # Dummy activation right at program start: forces the Relu table
        # load (1.3us) to happen while the first label DMA is in flight
        # instead of stalling the first real ACT plane.
        warm = singles.tile([R, 1], mybir.dt.float32)
        nc.gpsimd.memset(warm[:], 0.0)
        act_prev = nc.scalar.activation(
            out=warm[:], in_=warm[:],
            func=mybir.ActivationFunctionType.Relu,
            bias=bias_t[:, 0:1], scale=1.0)

        # PREPARE_ONLY kv_writeback emitted early: reads the no-deps alias
        # so descriptor generation runs on Pool right away; the trigger at
        # the end fires the transfer with no HWDGE/DGE/desc-gen latency.
        ctx_idxs = singles.tile([R, 1], mybir.dt.int32)
        nc.gpsimd.memset(ctx_idxs[:], 0)
        dma_sem = nc.alloc_semaphore("out_dma")
        ot4 = ot_alias_h.ap()[:, :].rearrange("r (a b n) -> r a b n",
                                              a=1, b=1)
        prep = nc.gpsimd.kv_writeback(
            out_ap=out_d.ap()[:, :, :, :],
            in_ap=ot4,
            ctx_idxs_ap=ctx_idxs[:, :],
            prepare_only=True,
            sem=dma_sem,
        )
        # Drop the manual completion sem: Tile's sem pass owns OnUpdate[0]
        # of a FixedSemIncDMA prep (it becomes the DMASW lane sem that the
        # epilogue waits on); a caller sem there deadlocks the epilogue.
        prep.ins.sync_info.on_update = [
            u for u in prep.ins.sync_info.on_update
            if getattr(u, "ant_name", None) != "out_dma"
        ]

        acc = psum_pool.tile([128, 512], mybir.dt.float32)   # gram bank
        accc = psum_pool.tile([128, 512], mybir.dt.float32)  # counts bank
        ones8 = singles.tile([R, 2, M], mybir.dt.float8e4)
        nc.vector.memset(ones8[:], 1.0)

        pred_ap = pred_d.ap()
        lab_ap = lab_d.ap()

        # input DMA stream, one queue, in DMA_ORDER (gapless: every small
        # label piece is followed by a big pred chunk)
        chunk_sc = np.concatenate([[0], np.cumsum(CHUNKS)])
        for kind, idx in DMA_ORDER:
            if kind == "lab":
                q0, q1 = LAB_PIECES[idx]
                nc.sync.dma_start(out=lab_u8[:, q0:q1], in_=lab_ap[:, q0:q1])
            else:
                s0, s1 = int(chunk_sc[idx]), int(chunk_sc[idx + 1])
                nc.sync.dma_start(out=pred_t[:, s0:s1, :, :],
                                  in_=pred_ap[:, s0:s1, :, :])

        # hinge planes, chained per engine to pin execution order
        prev = {"dve": None, "act": act_prev, "pool": None}
        for eng, i, s0, s1 in PLANE_ASSIGN:
            oh_slice = oh8[:, s0:s1, i, :]
            lab_slice = lab_u8[:, s0 * J:s1 * J].rearrange(
                "r (s j) -> r s j", j=J)
            if eng == "dve":
                h = nc.vector.tensor_scalar(
                    out=oh_slice, in0=lab_slice,
                    scalar1=float(i) + 0.5, scalar2=0.0,
                    op0=mybir.AluOpType.subtract, op1=mybir.AluOpType.max)
            elif eng == "act":
                ci = bias_col[i]
                h = nc.scalar.activation(
                    out=oh_slice, in_=lab_slice,
                    func=mybir.ActivationFunctionType.Relu,
                    bias=bias_t[:, ci:ci + 1], scale=1.0)
            else:
                h = nc.gpsimd.tensor_scalar(
                    out=oh_slice, in0=lab_slice,
                    scalar1=float(i) + 0.5, scalar2=0.0,
                    op0=mybir.AluOpType.subtract, op1=mybir.AluOpType.max)
            if prev[eng] is not None:
                add_dep_helper(_raw(h), _raw(prev[eng]), False,
                               "serialize plane groups")
            prev[eng] = h

        # warmup matmuls (optional)
        if N_WARMUP:
            dw = singles.tile([R, M], mybir.dt.bfloat16)
            dr_ = singles.tile([R, N], mybir.dt.bfloat16)
            scratch = psum_pool.tile([128, N], mybir.dt.float32)
            nc.vector.memset(dw[:], 0.0)
            nc.vector.memset(dr_[:], 0.0)
            for _ in range(N_WARMUP):
                nc.tensor.matmul(scratch[:, :], lhsT=dw[:], rhs=dr_[:],
                                 start=True, stop=True, skip_group_check=True)

        # gram + counts matmuls: fp8 DoubleRow, two superchunks each.
        # The counts matmul (all-ones weights) accumulates per-plane column
        # sums -> exact pixel counts, replacing per-instruction accum_out
        # (which GPSIMD doesn't support in hardware).
        for t in range(NSC // 2):
            s = 2 * t
            nc.tensor.matmul(
                acc[:, :N],
                lhsT=pred_t[:, s:s + 2, :, :],
                rhs=oh8[:, s:s + 2, :, :],
                start=(t == 0), stop=(t == NSC // 2 - 1),
                perf_mode=mybir.MatmulPerfMode.DoubleRow,
                skip_group_check=True,
            )
            nc.tensor.matmul(
                accc[:, :N],
                lhsT=ones8[:, :, :],
                rhs=oh8[:, s:s + 2, :, :],
                start=(t == 0), stop=(t == NSC // 2 - 1),
                perf_mode=mybir.MatmulPerfMode.DoubleRow,
                skip_group_check=True,
            )

        # gram -> output tile next to the counts.  The output ships via a
        # PREPARE_ONLY kv_writeback: descriptor generation runs on Pool as
        # soon as its queue drains (addresses only — Tile defers the RAW
        # deps on `ot` to the trigger), so the tail pays no HWDGE/DGE
        # latency; the trigger just fires the DMA engines.  The nop holds
        # Pool until the DMA completion sem so the epilogue barrier covers
        # the transfer.
        # gram -> output tile, then fire the prepared writeback.  The
        # trigger must not run before `ot` is complete: cross-engine waits
        # sit on a Pool engine_nop (engine-stage waits leave Pool SEQ
        # free), order-chained behind Pool's planes; the trigger sync-deps
        # on the nop's same-engine tick.
        copy_h = nc.vector.tensor_copy(out=ot[:, :N], in_=acc[:, :N])
        copy_c = nc.scalar.copy(out=ot[:, N:], in_=accc[:, :N])
        gate = nc.gpsimd.engine_nop()
        add_dep_helper(_raw(gate), _raw(prev["pool"]), False,
                       "gate after pool planes")
        add_dep_helper(_raw(gate), _raw(copy_h), True, "out gram ready")
        add_dep_helper(_raw(gate), _raw(copy_c), True, "out counts ready")
        trig = nc.gpsimd.trigger_dma(count=1)
        add_dep_helper(_raw(trig), _raw(gate), True, "fire after gate")

    nc.compile()
    return nc


def _get_program():
    global _cached_nc
    if _cached_nc is None:
        _cached_nc = _build_program()
    return _cached_nc


def _make_in_maps(pred_similarities, kernel_mask_ndi_labels):
    import ml_dtypes

    pred = (
        np.asarray(pred_similarities, dtype=np.float32)
        .reshape(B, C, R, NSC, J)
        .astype(ml_dtypes.float8_e4m3fn)
    )
    predperm = np.ascontiguousarray(pred.transpose(0, 2, 3, 1, 4))
    lab = np.asarray(kernel_mask_ndi_labels).reshape(B, R, Q).astype(np.uint8)
    return [{"pred": predperm[b], "lab": lab[b]} for b in range(B)]


def _hinge_T():
    # T[i, k-1] = relu(k - (i+0.5)), unknowns s[k], k = 1..8
    i = np.arange(K)[:, None]
    k = np.arange(1, K + 1)[None, :]
    return np.maximum(0.0, k - (i + 0.5))


def _finalize(results):
    f_sigma = float(np.log(SIGMA**2 + 1.0))
    T = _hinge_T()
    total = 0.0
    for b in range(B):
        O = np.asarray(results[b]["out"], dtype=np.float64).reshape(R, OUTW)
        gram = O[:, :N].reshape(C, J, K, J)
        u = np.einsum("cjij->ic", gram)              # [plane i, c]
        A = O[0, N:].reshape(K, J).sum(axis=1)       # per-plane pixel sums
        s = np.linalg.solve(T, u)                    # [k, c], labels 1..8
        n = np.linalg.solve(T, A)                    # [k] counts
        present = np.nonzero(n > 0.5)[0]
        num_kernel = int(present.max()) + 1 if present.size else 0
        m = float(num_kernel)
        snorm = np.sqrt((s * s).sum(axis=1))
        f = np.log(np.maximum(SIGMA - snorm, 0.0) ** 2 + 1.0)
        valid = np.arange(1, K + 1) <= num_kernel
        per_kernel = float((n * (f - f_sigma))[valid].sum())
        num_pairs = m * (m - 1.0) * 0.5
        total += (m - 1.0) * per_kernel + num_pairs * (B * P_PIX) * f_sigma
    return np.asarray(total, dtype=np.float32)


def kernel(pred_similarities, kernel_mask_ndi_labels):
    nc = _get_program()
    in_maps = _make_in_maps(pred_similarities, kernel_mask_ndi_labels)
    last_err = None
    for attempt in range(4):
        try:
            res = run_bass_kernel_spmd(nc, in_maps, core_ids=list(range(B)))
            # materialize inside the try: device errors can surface lazily
            results = [
                {k: np.asarray(v) for k, v in r.items()} for r in res.results
            ]
            return _finalize(results)
        except Exception as e:  # noqa: BLE001 - retry transient device wedges
            last_err = e
            import time

            time.sleep(10 * (attempt + 1))
    raise last_err


def modeled_exec_time_ns():
    from concourse.timeline_sim import TimelineSim

    return TimelineSim(_get_program(), trace=False).simulate()


# revision 40
# speedup vs baseline: 1.1953x; 1.0467x over previous
"""Trainium2 Bass kernel for nn_DiscriminationLoss (segment_reduce), v5.

Per core (one image, data-parallel over batch): segment sums
  s[k, c] = sum_p pred[p, c] * [lab[p] == k], k = 1..8, plus counts n[k].

v5 key ideas (vs v4 baseline's one-hot + bf16/fp8 mix at 18.7us):
  * Hinge basis instead of one-hot: plane_i(p) = relu(lab[p] - (i+0.5)),
    i = 0..7.  Linear in the one-hot with an invertible 8x8 matrix
    T[i,k] = relu(k - i - 0.5) (k = 1..8); background (lab=0) maps to 0 in
    every plane.  The host solves T s~ = u after the fact.  Unlike
    is_equal, a hinge plane is a single pass on ANY engine:
      - DVE:  tensor_scalar(op0=subtract, op1=max)
      - ACT:  activation(Relu, bias=-(i+0.5))
      - Pool: gpsimd tensor_scalar
    so plane generation is split across all three (DVE ~4 planes,
    ACT ~2.5, Pool ~1.5), each comfortably under the DMA wall.
  * All planes fp8 (values i+0.5 are exact in e4m3) -> every matmul runs
    fp8 DoubleRow: PE work is 100 x 64cy = tiny.
  * Counts ride on accum_out of each plane instruction (free in the cost
    model, exact f32 sums of halves); A[i] = sum T[i,k] n_k, host solves.
  * One label DMA (big contiguous pieces, no sub-512B descriptor
    penalty), 8 pred chunk DMAs all >= 627ns so HWDGE pipelining never
    gaps the DMA engines.
  * Counts are copied into spare columns of the gram PSUM bank so ONE
    output DMA ships everything.

Cost model floor: preamble 0.7 + first-DMA stages 1.3 + DMA busy 10.5
(lab 1.14 + pred 9.10 + out 0.23) + tail (mm+sem+stages+xfer+sem+barrier
~3.0) ~= 15.3us.
"""

import numpy as np
from contextlib import ExitStack

import concourse.bass as bass  # noqa: F401
import concourse.tile as tile
from concourse import bacc, mybir
from concourse.bass_utils import run_bass_kernel_spmd
from concourse.tile_rust import add_dep_helper

B, C, H, W = 8, 8, 640, 640
P_PIX = H * W
R = 128
Q = P_PIX // R         # 3200
SIGMA = 3.0
J = 16
K = 8                  # number of hinge planes (= number of kernels)
M = C * J              # 128
N = K * J              # 128
NSC = Q // J           # 200

# pred DMA chunks (superchunks each): all even (DoubleRow pairs never
# straddle a chunk) and big enough that each transfer covers the next
# DMA instruction's 627ns HWDGE stage, so the DMA engines never gap.
# The tiny final chunk lets most of the last matmuls start one DMA-sem
# (900ns) earlier.
CHUNKS = [26, 26, 26, 26, 24, 24, 24, 12, 8, 4]
assert sum(CHUNKS) == NSC

# label DMA pieces in Q columns (16 per superchunk); interleaved between
# the first pred chunks (big transfers cover the small pieces' HWDGE
# time).  Piece 1 (648ns) covers plane spans up to sc 114 and exactly
# bridges the first pred chunk's HWDGE+DGE latency (no DMA gap).
LAB_PIECES = [(0, 1824), (1824, 3200)]
# DMA program: ("lab", piece_idx) / ("pred", chunk_idx) in stream order
DMA_ORDER = [("lab", 0), ("pred", 0), ("lab", 1)] + [
    ("pred", i) for i in range(1, len(CHUNKS))]

# plane assignment: ordered per engine: list of (engine, plane_i, sc0, sc1)
# DVE: planes 0-3 + tail of 6; ACT: planes 4,5 + middle of 6; Pool:
# plane 7 + head of 6.  Early superchunks get small spans on every engine
# so the matmul stream starts ~5us.
# 4-stage lockstep ladder: each stage covers 50 superchunks; within a
# stage DVE makes planes 0-3 (+ head of 6), ACT planes 4-5, Pool plane 7
# (+ tail of 6).  Stage k's planes complete ~0.5-1us before stage k's
# pred chunks clear their DMA sems, so the matmul stream stays DMA-paced.
PLANE_ASSIGN = []
for s0, s1 in [(0, 50), (50, 100), (100, 150), (150, 200)]:
    sm = s0 + 14
    for i in range(4):
        PLANE_ASSIGN.append(("dve", i, s0, s1))
    PLANE_ASSIGN.append(("dve", 6, s0, sm))
    PLANE_ASSIGN.append(("act", 4, s0, s1))
    PLANE_ASSIGN.append(("act", 5, s0, s1))
    PLANE_ASSIGN.append(("pool", 7, s0, s1))
    PLANE_ASSIGN.append(("pool", 6, sm, s1))

OUTW = 2 * N               # [gram | counts] in the single output

# sanity: every (plane, sc) covered exactly once
_cover = np.zeros((K, NSC), dtype=int)
for _, i, s0, s1 in PLANE_ASSIGN:
    _cover[i, s0:s1] += 1
assert (_cover == 1).all()

N_WARMUP = 0               # PE keeps pace even at mid p-state

_cached_nc = None


def _raw(h):
    return getattr(h, "ins", h)


def _build_program():
    nc = bacc.Bacc("TRN2", target_bir_lowering=False, debug=False,
                   enable_asserts=False, num_devices=B)
    pred_d = nc.dram_tensor("pred", [R, NSC, C, J], mybir.dt.float8e4,
                            kind="ExternalInput")
    lab_d = nc.dram_tensor("lab", [R, Q], mybir.dt.uint8,
                           kind="ExternalInput")
    # output shaped for kv_writeback: [batch=1, dhi=128, dho=1, n_ctx=OUTW]
    out_d = nc.dram_tensor("out", [1, R, 1, OUTW], mybir.dt.float32,
                           kind="ExternalOutput")

    with tile.TileContext(nc) as tc, ExitStack() as ctx:
        singles = ctx.enter_context(tc.tile_pool(name="singles", bufs=1))
        psum_pool = ctx.enter_context(
            tc.tile_pool(name="psum", bufs=1, space="PSUM"))

        pred_t = singles.tile([R, NSC, C, J], mybir.dt.float8e4)
        oh8 = singles.tile([R, NSC, K, J], mybir.dt.float8e4)
        lab_u8 = singles.tile([R, Q], mybir.dt.uint8)
        # output staging tile + an alias at the same bytes: the writeback
        # PREP reads the alias so Tile attaches no data deps to it (the
        # trigger is gated manually); writers use `ot`.
        ot_h = nc.alloc_sbuf_tensor("ot", [R, OUTW], mybir.dt.float32)
        ot_alias_h = nc.alloc_sbuf_tensor_at(
            "ot_alias", [R, OUTW], mybir.dt.float32,
            offset=nc.lookup_mloc(ot_h).addr)
        ot = ot_h.ap()
        # biases for the ACT planes (pass scalar directly on DVE/Pool);
        # bias_t[:, i] = -(i + 0.5) for the planes ACT owns, plus a zero
        # column for the table-warm dummy activation.
        act_planes = sorted({i for e, i, _, _ in PLANE_ASSIGN if e == "act"})
        bias_t = singles.tile([R, len(act_planes) + 1], mybir.dt.float32)
        bias_col = {}
        nc.gpsimd.memset(bias_t[:, 0:1], 0.0)
        for ci, i in enumerate(act_planes):
            bias_col[i] = ci + 1
            nc.gpsimd.memset(bias_t[:, ci + 1:ci + 2], -(float(i) + 0.5))

        # Dummy activation right at program start: forces the Relu table
        # load (1.3us) to happen while the first label DMA is in flight
        # instead of stalling the first real ACT plane.
        warm = singles.tile([R, 1], mybir.dt.float32)
        nc.gpsimd.memset(warm[:], 0.0)
        act_prev = nc.scalar.activation(
            out=warm[:], in_=warm[:],
            func=mybir.ActivationFunctionType.Relu,
            bias=bias_t[:, 0:1], scale=1.0)

        # PREPARE_ONLY kv_writeback emitted early: reads the no-deps alias
        # so descriptor generation runs on Pool right away; the trigger at
        # the end fires the transfer with no HWDGE/DGE/desc-gen latency.
        ctx_idxs = singles.tile([R, 1], mybir.dt.int32)
        nc.gpsimd.memset(ctx_idxs[:], 0)
        dma_sem = nc.alloc_semaphore("out_dma")
        ot4 = ot_alias_h.ap()[:, :].rearrange("r (a b n) -> r a b n",
                                              a=1, b=1)
        prep = nc.gpsimd.kv_writeback(
            out_ap=out_d.ap()[:, :, :, :],
            in_ap=ot4,
            ctx_idxs_ap=ctx_idxs[:, :],
            prepare_only=True,
            sem=dma_sem,
        )
        # Drop the manual completion sem: Tile's sem pass owns OnUpdate[0]
        # of a FixedSemIncDMA prep (it becomes the DMASW lane sem that the
        # epilogue waits on); a caller sem there deadlocks the epilogue.
        prep.ins.sync_info.on_update = [
            u for u in prep.ins.sync_info.on_update
            if getattr(u, "ant_name", None) != "out_dma"
        ]

        acc = psum_pool.tile([128, 512], mybir.dt.float32)   # gram bank
        accc = psum_pool.tile([128, 512], mybir.dt.float32)  # counts bank
        ones8 = singles.tile([R, 2, M], mybir.dt.float8e4)
        nc.vector.memset(ones8[:], 1.0)

        pred_ap = pred_d.ap()
        lab_ap = lab_d.ap()

        # input DMA stream, one queue, in DMA_ORDER (gapless: every small
        # label piece is followed by a big pred chunk)
        chunk_sc = np.concatenate([[0], np.cumsum(CHUNKS)])
        for kind, idx in DMA_ORDER:
            if kind == "lab":
                q0, q1 = LAB_PIECES[idx]
                nc.sync.dma_start(out=lab_u8[:, q0:q1], in_=lab_ap[:, q0:q1])
            else:
                s0, s1 = int(chunk_sc[idx]), int(chunk_sc[idx + 1])
                nc.sync.dma_start(out=pred_t[:, s0:s1, :, :],
                                  in_=pred_ap[:, s0:s1, :, :])

        # hinge planes, chained per engine to pin execution order
        prev = {"dve": None, "act": act_prev, "pool": None}
        for eng, i, s0, s1 in PLANE_ASSIGN:
            oh_slice = oh8[:, s0:s1, i, :]
            lab_slice = lab_u8[:, s0 * J:s1 * J].rearrange(
                "r (s j) -> r s j", j=J)
            if eng == "dve":
                h = nc.vector.tensor_scalar(
                    out=oh_slice, in0=lab_slice,
                    scalar1=float(i) + 0.5, scalar2=0.0,
                    op0=mybir.AluOpType.subtract, op1=mybir.AluOpType.max)
            elif eng == "act":
                ci = bias_col[i]
                h = nc.scalar.activation(
                    out=oh_slice, in_=lab_slice,
                    func=mybir.ActivationFunctionType.Relu,
                    bias=bias_t[:, ci:ci + 1], scale=1.0)
            else:
                h = nc.gpsimd.tensor_scalar(
                    out=oh_slice, in0=lab_slice,
                    scalar1=float(i) + 0.5, scalar2=0.0,
                    op0=mybir.AluOpType.subtract, op1=mybir.AluOpType.max)
            if prev[eng] is not None:
                add_dep_helper(_raw(h), _raw(prev[eng]), False,
                               "serialize plane groups")
            prev[eng] = h

        # warmup matmuls (optional)
        if N_WARMUP:
            dw = singles.tile([R, M], mybir.dt.bfloat16)
            dr_ = singles.tile([R, N], mybir.dt.bfloat16)
            scratch = psum_pool.tile([128, N], mybir.dt.float32)
            nc.vector.memset(dw[:], 0.0)
            nc.vector.memset(dr_[:], 0.0)
            for _ in range(N_WARMUP):
                nc.tensor.matmul(scratch[:, :], lhsT=dw[:], rhs=dr_[:],
                                 start=True, stop=True, skip_group_check=True)

        # gram + counts matmuls: fp8 DoubleRow, two superchunks each.
        # The counts matmul (all-ones weights) accumulates per-plane column
        # sums -> exact pixel counts, replacing per-instruction accum_out
        # (which GPSIMD doesn't support in hardware).  Counts matmuls only
        # depend on the planes, so they are emitted per 25-pair stage ahead
        # of that stage's pred-gated gram matmuls — the in-order PE queue
        # then never parks a counts matmul behind a DMA sem.
        STAGE_PAIRS = 25
        npair = NSC // 2
        for g in range(npair // STAGE_PAIRS):
            for t in range(g * STAGE_PAIRS, (g + 1) * STAGE_PAIRS):
                s = 2 * t
                nc.tensor.matmul(
                    accc[:, :N],
                    lhsT=ones8[:, :, :],
                    rhs=oh8[:, s:s + 2, :, :],
                    start=(t == 0), stop=(t == npair - 1),
                    perf_mode=mybir.MatmulPerfMode.DoubleRow,
                    skip_group_check=True,
                )
            for t in range(g * STAGE_PAIRS, (g + 1) * STAGE_PAIRS):
                s = 2 * t
                nc.tensor.matmul(
                    acc[:, :N],
                    lhsT=pred_t[:, s:s + 2, :, :],
                    rhs=oh8[:, s:s + 2, :, :],
                    start=(t == 0), stop=(t == npair - 1),
                    perf_mode=mybir.MatmulPerfMode.DoubleRow,
                    skip_group_check=True,
                )

        # gram -> output tile next to the counts.  The output ships via a
        # PREPARE_ONLY kv_writeback: descriptor generation runs on Pool as
        # soon as its queue drains (addresses only — Tile defers the RAW
        # deps on `ot` to the trigger), so the tail pays no HWDGE/DGE
        # latency; the trigger just fires the DMA engines.  The nop holds
        # Pool until the DMA completion sem so the epilogue barrier covers
        # the transfer.
        # gram -> output tile, then fire the prepared writeback.  The
        # trigger must not run before `ot` is complete: cross-engine waits
        # sit on a Pool engine_nop (engine-stage waits leave Pool SEQ
        # free), order-chained behind Pool's planes; the trigger sync-deps
        # on the nop's same-engine tick.
        copy_h = nc.vector.tensor_copy(out=ot[:, :N], in_=acc[:, :N])
        copy_c = nc.scalar.copy(out=ot[:, N:], in_=accc[:, :N])
        trig = nc.gpsimd.trigger_dma(count=1)
        add_dep_helper(_raw(trig), _raw(prev["pool"]), False,
                       "after pool planes")
        add_dep_helper(_raw(trig), _raw(copy_h), True, "out gram ready")
        add_dep_helper(_raw(trig), _raw(copy_c), True, "out counts ready")

    nc.compile()
    return nc


def _get_program():
    global _cached_nc
    if _cached_nc is None:
        _cached_nc = _build_program()
    return _cached_nc


def _make_in_maps(pred_similarities, kernel_mask_ndi_labels):
    import ml_dtypes

    pred = (
        np.asarray(pred_similarities, dtype=np.float32)
        .reshape(B, C, R, NSC, J)
        .astype(ml_dtypes.float8_e4m3fn)
    )
    predperm = np.ascontiguousarray(pred.transpose(0, 2, 3, 1, 4))
    lab = np.asarray(kernel_mask_ndi_labels).reshape(B, R, Q).astype(np.uint8)
    return [{"pred": predperm[b], "lab": lab[b]} for b in range(B)]


def _hinge_T():
    # T[i, k-1] = relu(k - (i+0.5)), unknowns s[k], k = 1..8
    i = np.arange(K)[:, None]
    k = np.arange(1, K + 1)[None, :]
    return np.maximum(0.0, k - (i + 0.5))


def _finalize(results):
    f_sigma = float(np.log(SIGMA**2 + 1.0))
    T = _hinge_T()
    total = 0.0
    for b in range(B):
        O = np.asarray(results[b]["out"], dtype=np.float64).reshape(R, OUTW)
        gram = O[:, :N].reshape(C, J, K, J)
        u = np.einsum("cjij->ic", gram)              # [plane i, c]
        A = O[0, N:].reshape(K, J).sum(axis=1)       # per-plane pixel sums
        s = np.linalg.solve(T, u)                    # [k, c], labels 1..8
        n = np.linalg.solve(T, A)                    # [k] counts
        present = np.nonzero(n > 0.5)[0]
        num_kernel = int(present.max()) + 1 if present.size else 0
        m = float(num_kernel)
        snorm = np.sqrt((s * s).sum(axis=1))
        f = np.log(np.maximum(SIGMA - snorm, 0.0) ** 2 + 1.0)
        valid = np.arange(1, K + 1) <= num_kernel
        per_kernel = float((n * (f - f_sigma))[valid].sum())
        num_pairs = m * (m - 1.0) * 0.5
        total += (m - 1.0) * per_kernel + num_pairs * (B * P_PIX) * f_sigma
    return np.asarray(total, dtype=np.float32)


def kernel(pred_similarities, kernel_mask_ndi_labels):
    nc = _get_program()
    in_maps = _make_in_maps(pred_similarities, kernel_mask_ndi_labels)
    last_err = None
    for attempt in range(4):
        try:
            res = run_bass_kernel_spmd(nc, in_maps, core_ids=list(range(B)))
            # materialize inside the try: device errors can surface lazily
            results = [
                {k: np.asarray(v) for k, v in r.items()} for r in res.results
            ]
            return _finalize(results)
        except Exception as e:  # noqa: BLE001 - retry transient device wedges
            last_err = e
            import time

            time.sleep(10 * (attempt + 1))
    raise last_err


def modeled_exec_time_ns():
    from concourse.timeline_sim import TimelineSim

    return TimelineSim(_get_program(), trace=False).simulate()
